# revision 16
# baseline (speedup 1.0000x reference)
"""Bilinear MoE-routing scores on 8 Trainium2 NeuronCores.

Problem: scores[n,k] = u[n,:] @ W_k @ v[n,:]; out[n] = sigmoid(scores[n, type_idx[n]]).
N=131072, D=256, K=8.

Sharding: rows grouped BY TYPE — core k gets exactly the rows with
type_idx == k, so each core runs one plain bilinear kernel against its own
W_k (8x less matmul work than data-parallel all-K). Host argsorts, pads each
group to a common n_pad, and scatters per-core results back to row order.

Precision: pure fp16 for u, v, AND W (validated offline on the exact
key=0 inputs: 6.0e-3 absmax output error vs the 2e-2 gate — fp32 PSUM
accumulation keeps products exact, only input rounding contributes).
This halves DMA bytes vs fp16-hi/lo + fp32-v: 1 KiB/row -> ~17 MB/core.

Device kernel (per core, SPMD):
  t[n,e] = sum_d uT[d,n] * W[d,e]   TensorE, fp16 x fp16 -> fp32 PSUM,
                                    2 matmuls per 128-row tile (~28 us)
  s[n]   = sum_e t[n,e] * v[n,e]    VectorE: one fused mul+cumsum custom op
                                    (MUL_CUMSUM_ANT, registered at import)
                                    per 8-tile PSUM supergroup; per-tile sums
                                    recovered as differences of the running
                                    sum at tile boundaries (ScalarE extracts
                                    the boundary column, VectorE diffs).
                                    ~37 us vs ~51 us for per-tile
                                    affine_mul_reduce (PSUM-source DVE ops run
                                    1x with a 120-cycle per-instruction
                                    penalty; the cumsum amortizes it 8x).
  out[n] = sigmoid(s[n])            ScalarE LUT

Measured (8 cores concurrent, paired-delta slope over 1024 on-device
iterations): ~47 us best / ~55-65 us typical per invocation vs ~46 us
for a DMA-only ablation of the same traffic — i.e. compute is fully
hidden and the kernel sits at the HBM roofline (16.9 MB/core at the
measured ~366 GB/s/NC effective HBM rate). Baseline (fp16-hi/lo u +
fp32 v, per-tile affine_mul_reduce): ~119 us.
"""

import math

import numpy as np

P = 128  # SBUF partitions
D = 256  # hidden dim
N_CORES = 8
CHUNK = 2048  # rows per DMA chunk (multiple of 128)
UBUFS = 3
VBUFS = 3
G = 8  # row-tiles per PSUM super-tile ([128, G*256] f32 = G/2 banks)
SCAN_MODE = True  # fused mul+cumsum custom DVE op vs per-tile affine_mul_reduce
STAGGER = False  # staggered semaphore reset in the benchmark repeat loop (crashes HW)

_PROGRAM_CACHE: dict = {}
_SCAN_OP = None


def _get_scan_op():
    """Register (once) and return the MUL_CUMSUM_ANT custom DVE op:
    out[p, k] = sum_{j<=k} in0[p, j] * in1[p, j]  (fp32 internal state).

    One fused 1x-rate pass replaces the per-tile affine_mul_reduce calls;
    per-instruction overhead is amortized over G row-tiles."""
    global _SCAN_OP
    if _SCAN_OP is not None:
        return _SCAN_OP
    import concourse.dve_ops as dve_ops
    from concourse.dve_spec import Spec, Src0, Src1, AluOp, scan, lower
    from concourse.dve_uop import DveOpSpec

    for o in dve_ops.OPS:
        if o.name == "MUL_CUMSUM_ANT":
            _SCAN_OP = o
            return o

    def _ref(in0, in1, s0, s1, imm2):
        return np.cumsum(
            in0.astype(np.float32) * np.asarray(in1, dtype=np.float32),
            axis=-1,
            dtype=np.float32,
        )

    spec = Spec(body=scan(AluOp.ADD, Src0 * Src1), reference=_ref)
    shas = {}
    for ver in ("v3", "v4"):
        uops = lower(spec, ver=ver)
        shas[ver] = DveOpSpec(
            name="MUL_CUMSUM_ANT", opcode=0, uops=uops, rd1_en=True
        ).sha(ver)
    op = dve_ops.DveOp("MUL_CUMSUM_ANT", spec, subdim=False, uops_sha=shas)
    dve_ops.OPS.append(op)
    dve_ops.CUSTOM_DVE_SPECS[op.name] = spec
    dve_ops._SUB_OPCODE_FOR_NAME[op.name] = (
        dve_ops._CUSTOM_DVE_ROW_BASE + len(dve_ops.OPS) - 1
    )
    _SCAN_OP = op
    return op


def _chunk_sizes(n_pad: int):
    """Small chunks at both ends for pipeline fill/drain, CHUNK in the middle.
    All sizes are multiples of 128; sum == n_pad."""
    rem = n_pad
    up = []
    for s in (512, 1024):
        if rem >= s + 1536 + CHUNK:
            up.append(s)
            rem -= s
    down = []
    for s in (1024, 512):
        if rem >= s + 512:
            down.append(s)
            rem -= s
    n_mid = rem // CHUNK
    leftover = rem - n_mid * CHUNK
    sizes = up + [CHUNK] * n_mid + ([leftover] if leftover else []) + down
    assert sum(sizes) == n_pad and all(s % P == 0 for s in sizes)
    return sizes


def _build_program(n_pad: int, repeat: int = 1, mode: str = "full"):
    """Build + compile the SPMD Bass program for n_pad rows per core.

    repeat > 1 wraps the body in an on-device loop (benchmarking only).
    mode: 'full' | 'no_dve' | 'no_pe' | 'dma_only' (ablation benches)."""
    import contextlib

    import concourse.bass as bass  # noqa: F401
    import concourse.mybir as mybir
    import concourse.tile as tile
    from concourse import bacc

    do_pe = mode in ("full", "no_dve")
    do_dve = mode in ("full", "no_pe")
    do_dma = mode != "empty"

    f32 = mybir.dt.float32
    f16 = mybir.dt.float16
    n_tiles = n_pad // P
    assert n_pad % P == 0
    scan_op = _get_scan_op() if SCAN_MODE else None
    # number of supergroups (for the boundary-column buffer)
    n_groups = sum(
        len(range(0, ch // P, G)) for ch in _chunk_sizes(n_pad)
    )

    nc = bacc.Bacc(
        "TRN2", target_bir_lowering=False, debug=False, num_devices=N_CORES
    )
    # u transposed + split into contraction halves: u_t[p, h, n] = u[n, h*128+p]
    u_t = nc.dram_tensor("u_t", [P, 2, n_pad], f16, kind="ExternalInput").ap()
    # v pre-permuted per row-tile: v_p[p, t, e] = v[t*128+p, e]
    v_p = nc.dram_tensor("v_p", [P, n_tiles, D], f16, kind="ExternalInput").ap()
    # w pre-permuted: w_p[p, h, e] = W[h*128+p, e]
    w_p = nc.dram_tensor("w_p", [P, 2, D], f16, kind="ExternalInput").ap()
    out = nc.dram_tensor("out", [n_pad], f32, kind="ExternalOutput").ap()

    with tile.TileContext(nc) as tc:
        with (
            tc.tile_pool(name="singles", bufs=1) as singles,
            tc.tile_pool(name="upool", bufs=UBUFS) as upool,
            tc.tile_pool(name="vpool", bufs=VBUFS) as vpool,
            tc.tile_pool(name="ppool", bufs=max(1, 16 // G), space="PSUM") as ppool,
            tc.tile_pool(name="psingles", bufs=1, space="PSUM") as psingles,
            tc.tile_pool(name="spool", bufs=2) as spool,
        ):
            rep_ctx = (
                tc.For_i(
                    0,
                    repeat,
                    1,
                    hint_engines=(
                        mybir.EngineType.PE,
                        mybir.EngineType.DVE,
                        mybir.EngineType.Activation,
                    ),
                    staggered_reset=STAGGER,
                )
                if repeat > 1
                else contextlib.nullcontext()
            )

            # s_buf[p, t] = score of padded row t*128+p
            s_buf = singles.tile([P, n_tiles], f32)
            sig_buf = singles.tile([P, n_tiles], f32)
            w_sb = singles.tile([P, 2, D], f16)
            nc.scalar.dma_start(out=w_sb, in_=w_p)
            if SCAN_MODE:
                # ends[p, gi, 0] = 0; ends[p, gi, 1+j] = running sum at end of
                # tile j of supergroup gi.  s[tile j] = ends[j+1] - ends[j].
                ends = singles.tile([P, n_groups, G + 1], f32)
                nc.vector.memset(ends, 0.0)
            static_ps = None
            if do_dve and not do_pe:
                # ablation: pre-zeroed PSUM tiles so the DVE reads allocated
                # data without any PE work inside the loop
                sps0 = psingles.tile([P, G, D], f32, tag="sps0")
                sps1 = psingles.tile([P, G, D], f32, tag="sps1")
                static_ps = [sps0, sps1]
                for sp in static_ps:
                    nc.vector.memset(sp, 0.0)

            with rep_ctx:
                if not do_dma:
                    # empty-loop overhead probe: one tiny DVE op per iteration
                    nc.vector.memset(s_buf[:, 0:1], 0.0)
                c0 = 0
                gi = 0
                for ch in _chunk_sizes(n_pad) if do_dma else []:
                    cht = ch // P
                    u_ch = upool.tile([P, 2, ch], f16, tag="u")
                    nc.sync.dma_start(out=u_ch, in_=u_t[:, :, c0 : c0 + ch])
                    t0 = c0 // P
                    v_ch = vpool.tile([P, cht, D], f16, tag="v")
                    nc.scalar.dma_start(out=v_ch, in_=v_p[:, t0 : t0 + cht, :])

                    for st in range(0, cht, G):
                        g = min(G, cht - st)
                        if do_pe:
                            ps = ppool.tile([P, g, D], f32, tag="ps")
                            for j in range(g):
                                sl = slice((st + j) * P, (st + j + 1) * P)
                                nc.tensor.matmul(
                                    ps[:, j, :], u_ch[:, 0, sl], w_sb[:, 0, :],
                                    start=True, stop=False,
                                )
                                nc.tensor.matmul(
                                    ps[:, j, :], u_ch[:, 1, sl], w_sb[:, 1, :],
                                    start=False, stop=True,
                                )
                        elif do_dve:
                            ps = static_ps[gi % 2][:, :g, :]
                        gt = t0 + st
                        if do_dve and SCAN_MODE:
                            cum = spool.tile([P, g, D], f32, tag="cum")
                            nc.vector._custom_dve(
                                scan_op,
                                out=cum.rearrange("p g d -> p (g d)"),
                                in0=ps.rearrange("p g d -> p (g d)"),
                                in1=v_ch[:, st : st + g, :].rearrange(
                                    "p g d -> p (g d)"
                                ),
                            )
                            # boundary column -> ends[:, gi, 1:1+g] (ScalarE)
                            nc.scalar.copy(
                                out=ends[:, gi, 1 : 1 + g],
                                in_=cum[:, :, D - 1 : D],
                            )
                            # per-tile sums = adjacent differences (VectorE)
                            nc.vector.tensor_tensor(
                                out=s_buf[:, gt : gt + g],
                                in0=ends[:, gi, 1 : 1 + g],
                                in1=ends[:, gi, 0:g],
                                op=mybir.AluOpType.subtract,
                            )
                        elif do_dve:
                            scr = spool.tile([P, g, D], f32, tag="scr")
                            for j in range(g):
                                nc.vector.affine_mul_reduce(
                                    out=scr[:, j, :],
                                    accum_out=s_buf[:, gt + j : gt + j + 1],
                                    in0=ps[:, j, :],
                                    in1=v_ch[:, st + j, :],
                                    scale=1.0,
                                    bias=0.0,
                                )
                        gi += 1
                    c0 += ch

                # incremental sigmoid + output drain
                if do_dve:
                    out_pt = out.rearrange("(p t) -> p t", p=P)
                    n_blk = 4
                    bnd = [round(i * n_tiles / n_blk) for i in range(n_blk + 1)]
                    for b0, b1 in zip(bnd[:-1], bnd[1:]):
                        if b1 > b0:
                            nc.scalar.activation(
                                out=sig_buf[:, b0:b1],
                                in_=s_buf[:, b0:b1],
                                func=mybir.ActivationFunctionType.Sigmoid,
                            )
                            nc.sync.dma_start(
                                out=out_pt[:, b0:b1], in_=sig_buf[:, b0:b1]
                            )

    nc.compile()
    return nc


def _get_program(n_pad: int):
    if n_pad not in _PROGRAM_CACHE:
        _PROGRAM_CACHE[n_pad] = _build_program(n_pad)
    return _PROGRAM_CACHE[n_pad]


def _prep(u, v, weights, type_idx):
    """Group rows by type, pad, cast fp16, build per-core input maps."""
    u = np.ascontiguousarray(np.asarray(u, dtype=np.float32))
    v = np.ascontiguousarray(np.asarray(v, dtype=np.float32))
    weights = np.ascontiguousarray(np.asarray(weights, dtype=np.float32))
    ti = np.asarray(type_idx).astype(np.int64).ravel()

    n, d = u.shape
    k = weights.shape[0]
    assert d == D and k == N_CORES

    order = np.argsort(ti, kind="stable")
    counts = np.bincount(ti, minlength=k)
    offsets = np.concatenate(([0], np.cumsum(counts)))
    n_pad = max(P, int(math.ceil(counts.max() / P)) * P)
    n_tiles = n_pad // P

    u16 = u.astype(np.float16)
    v16 = v.astype(np.float16)

    in_maps = []
    core_rows = []
    for c in range(N_CORES):
        rows = order[offsets[c] : offsets[c + 1]]
        core_rows.append(rows)
        cnt = len(rows)
        # u_t[p, h, n] = u[n, h*128+p]
        u_t = np.zeros((P, 2, n_pad), dtype=np.float16)
        ut = u16[rows].T.reshape(2, P, cnt)  # [h, p, n]
        u_t[:, :, :cnt] = ut.transpose(1, 0, 2)
        # v_p[p, t, e] = v[t*128+p, e]
        v_pad = np.zeros((n_pad, D), dtype=np.float16)
        v_pad[:cnt] = v16[rows]
        v_pc = v_pad.reshape(n_tiles, P, D).transpose(1, 0, 2)
        # w_p[p, h, e] = W[h*128+p, e]
        w16 = weights[c].astype(np.float16)
        w_pc = w16.reshape(2, P, D).transpose(1, 0, 2)
        in_maps.append(
            {
                "u_t": u_t,
                "v_p": np.ascontiguousarray(v_pc),
                "w_p": np.ascontiguousarray(w_pc),
            }
        )
    return in_maps, core_rows, n_pad


def _run(u, v, weights, type_idx, trace=False):
    from concourse import bass_utils
    from concourse.bass_interp import get_hw_module

    n = np.asarray(u).shape[0]
    in_maps, core_rows, n_pad = _prep(u, v, weights, type_idx)
    n_tiles = n_pad // P

    nc = _get_program(n_pad)
    old_m = nc.m
    nc.m = get_hw_module(nc.m)
    try:
        res = bass_utils.run_bass_kernel_spmd(
            nc, in_maps, core_ids=list(range(N_CORES)), trace=trace
        )
    finally:
        nc.m = old_m

    final = np.empty((n,), dtype=np.float32)
    for c in range(N_CORES):
        arr = np.asarray(res.results[c]["out"]).reshape(P, n_tiles)
        per_row = arr.T.reshape(-1)[: len(core_rows[c])]
        final[core_rows[c]] = per_row
    return final, res


def kernel(**inputs) -> np.ndarray:
    out, _ = _run(
        inputs["u_hidden"],
        inputs["v_hidden"],
        inputs["weights"],
        inputs["type_idx"],
        trace=False,
    )
    return out


# revision 18
# speedup vs baseline: 1.0049x; 1.0049x over previous
"""Bilinear MoE-routing scores on 8 Trainium2 NeuronCores.

Problem: scores[n,k] = u[n,:] @ W_k @ v[n,:]; out[n] = sigmoid(scores[n, type_idx[n]]).
N=131072, D=256, K=8.

Sharding: rows grouped BY TYPE — core k gets exactly the rows with
type_idx == k, so each core runs one plain bilinear kernel against its own
W_k (8x less matmul work than data-parallel all-K). Host argsorts, pads each
group to a common n_pad, and scatters per-core results back to row order.

Precision: pure fp16 for u, v, AND W (validated offline on the exact
key=0 inputs: 6.0e-3 absmax output error vs the 2e-2 gate — fp32 PSUM
accumulation keeps products exact, only input rounding contributes).
This halves DMA bytes vs fp16-hi/lo + fp32-v: 1 KiB/row -> ~17 MB/core.

Device kernel (per core, SPMD):
  t[n,e] = sum_d uT[d,n] * W[d,e]   TensorE, fp16 x fp16 -> fp32 PSUM,
                                    2 matmuls per 128-row tile (~28 us)
  s[n]   = sum_e t[n,e] * v[n,e]    VectorE: one fused mul+cumsum custom op
                                    (MUL_CUMSUM_ANT, registered at import)
                                    per 8-tile PSUM supergroup; per-tile sums
                                    recovered as differences of the running
                                    sum at tile boundaries (ScalarE extracts
                                    the boundary column, VectorE diffs).
                                    ~37 us vs ~51 us for per-tile
                                    affine_mul_reduce (PSUM-source DVE ops run
                                    1x with a 120-cycle per-instruction
                                    penalty; the cumsum amortizes it 8x).
  out[n] = sigmoid(s[n])            ScalarE LUT

Measured (8 cores concurrent, paired-delta slope over 1024 on-device
iterations): ~47 us best / ~55-67 us under ambient HBM contention, vs
~46 us for a DMA-only ablation of the same traffic — i.e. compute is
fully hidden and the kernel sits at the HBM roofline (16.9 MB/core at
the measured ~366 GB/s/NC effective HBM rate). CHUNK=1024 with 6 DMA
buffers won an interleaved A/B against 2048/3 and 2048/6 (ambient load
drifts minute-to-minute, so configs must be compared interleaved in one
process). Baseline (fp16-hi/lo u + fp32 v, per-tile affine_mul_reduce):
~119 us.
"""

import math

import numpy as np

P = 128  # SBUF partitions
D = 256  # hidden dim
N_CORES = 8
CHUNK = 1024  # rows per DMA chunk (multiple of 128)
UBUFS = 6  # deep prefetch rides through transient HBM-contention dips
VBUFS = 6
G = 8  # row-tiles per PSUM super-tile ([128, G*256] f32 = G/2 banks)
SCAN_MODE = True  # fused mul+cumsum custom DVE op vs per-tile affine_mul_reduce
STAGGER = False  # staggered semaphore reset in the benchmark repeat loop (crashes HW)

_PROGRAM_CACHE: dict = {}
_SCAN_OP = None


def _get_scan_op():
    """Register (once) and return the MUL_CUMSUM_ANT custom DVE op:
    out[p, k] = sum_{j<=k} in0[p, j] * in1[p, j]  (fp32 internal state).

    One fused 1x-rate pass replaces the per-tile affine_mul_reduce calls;
    per-instruction overhead is amortized over G row-tiles."""
    global _SCAN_OP
    if _SCAN_OP is not None:
        return _SCAN_OP
    import concourse.dve_ops as dve_ops
    from concourse.dve_spec import Spec, Src0, Src1, AluOp, scan, lower
    from concourse.dve_uop import DveOpSpec

    for o in dve_ops.OPS:
        if o.name == "MUL_CUMSUM_ANT":
            _SCAN_OP = o
            return o

    def _ref(in0, in1, s0, s1, imm2):
        return np.cumsum(
            in0.astype(np.float32) * np.asarray(in1, dtype=np.float32),
            axis=-1,
            dtype=np.float32,
        )

    spec = Spec(body=scan(AluOp.ADD, Src0 * Src1), reference=_ref)
    shas = {}
    for ver in ("v3", "v4"):
        uops = lower(spec, ver=ver)
        shas[ver] = DveOpSpec(
            name="MUL_CUMSUM_ANT", opcode=0, uops=uops, rd1_en=True
        ).sha(ver)
    op = dve_ops.DveOp("MUL_CUMSUM_ANT", spec, subdim=False, uops_sha=shas)
    dve_ops.OPS.append(op)
    dve_ops.CUSTOM_DVE_SPECS[op.name] = spec
    dve_ops._SUB_OPCODE_FOR_NAME[op.name] = (
        dve_ops._CUSTOM_DVE_ROW_BASE + len(dve_ops.OPS) - 1
    )
    _SCAN_OP = op
    return op


def _chunk_sizes(n_pad: int):
    """Small chunks at both ends for pipeline fill/drain, CHUNK in the middle.
    All sizes are multiples of 128; sum == n_pad."""
    rem = n_pad
    up = []
    for s in (512, 1024):
        if rem >= s + 1536 + CHUNK:
            up.append(s)
            rem -= s
    down = []
    for s in (1024, 512):
        if rem >= s + 512:
            down.append(s)
            rem -= s
    n_mid = rem // CHUNK
    leftover = rem - n_mid * CHUNK
    sizes = up + [CHUNK] * n_mid + ([leftover] if leftover else []) + down
    assert sum(sizes) == n_pad and all(s % P == 0 for s in sizes)
    return sizes


def _build_program(n_pad: int, repeat: int = 1, mode: str = "full"):
    """Build + compile the SPMD Bass program for n_pad rows per core.

    repeat > 1 wraps the body in an on-device loop (benchmarking only).
    mode: 'full' | 'no_dve' | 'no_pe' | 'dma_only' (ablation benches)."""
    import contextlib

    import concourse.bass as bass  # noqa: F401
    import concourse.mybir as mybir
    import concourse.tile as tile
    from concourse import bacc

    do_pe = mode in ("full", "no_dve")
    do_dve = mode in ("full", "no_pe")
    do_dma = mode != "empty"

    f32 = mybir.dt.float32
    f16 = mybir.dt.float16
    n_tiles = n_pad // P
    assert n_pad % P == 0
    scan_op = _get_scan_op() if SCAN_MODE else None
    # number of supergroups (for the boundary-column buffer)
    n_groups = sum(
        len(range(0, ch // P, G)) for ch in _chunk_sizes(n_pad)
    )

    nc = bacc.Bacc(
        "TRN2", target_bir_lowering=False, debug=False, num_devices=N_CORES
    )
    # u transposed + split into contraction halves: u_t[p, h, n] = u[n, h*128+p]
    u_t = nc.dram_tensor("u_t", [P, 2, n_pad], f16, kind="ExternalInput").ap()
    # v pre-permuted per row-tile: v_p[p, t, e] = v[t*128+p, e]
    v_p = nc.dram_tensor("v_p", [P, n_tiles, D], f16, kind="ExternalInput").ap()
    # w pre-permuted: w_p[p, h, e] = W[h*128+p, e]
    w_p = nc.dram_tensor("w_p", [P, 2, D], f16, kind="ExternalInput").ap()
    out = nc.dram_tensor("out", [n_pad], f32, kind="ExternalOutput").ap()

    with tile.TileContext(nc) as tc:
        with (
            tc.tile_pool(name="singles", bufs=1) as singles,
            tc.tile_pool(name="upool", bufs=UBUFS) as upool,
            tc.tile_pool(name="vpool", bufs=VBUFS) as vpool,
            tc.tile_pool(name="ppool", bufs=max(1, 16 // G), space="PSUM") as ppool,
            tc.tile_pool(name="psingles", bufs=1, space="PSUM") as psingles,
            tc.tile_pool(name="spool", bufs=2) as spool,
        ):
            rep_ctx = (
                tc.For_i(
                    0,
                    repeat,
                    1,
                    hint_engines=(
                        mybir.EngineType.PE,
                        mybir.EngineType.DVE,
                        mybir.EngineType.Activation,
                    ),
                    staggered_reset=STAGGER,
                )
                if repeat > 1
                else contextlib.nullcontext()
            )

            # s_buf[p, t] = score of padded row t*128+p
            s_buf = singles.tile([P, n_tiles], f32)
            sig_buf = singles.tile([P, n_tiles], f32)
            w_sb = singles.tile([P, 2, D], f16)
            nc.scalar.dma_start(out=w_sb, in_=w_p)
            if SCAN_MODE:
                # ends[p, gi, 0] = 0; ends[p, gi, 1+j] = running sum at end of
                # tile j of supergroup gi.  s[tile j] = ends[j+1] - ends[j].
                ends = singles.tile([P, n_groups, G + 1], f32)
                nc.vector.memset(ends, 0.0)
            static_ps = None
            if do_dve and not do_pe:
                # ablation: pre-zeroed PSUM tiles so the DVE reads allocated
                # data without any PE work inside the loop
                sps0 = psingles.tile([P, G, D], f32, tag="sps0")
                sps1 = psingles.tile([P, G, D], f32, tag="sps1")
                static_ps = [sps0, sps1]
                for sp in static_ps:
                    nc.vector.memset(sp, 0.0)

            with rep_ctx:
                if not do_dma:
                    # empty-loop overhead probe: one tiny DVE op per iteration
                    nc.vector.memset(s_buf[:, 0:1], 0.0)
                c0 = 0
                gi = 0
                for ch in _chunk_sizes(n_pad) if do_dma else []:
                    cht = ch // P
                    u_ch = upool.tile([P, 2, ch], f16, tag="u")
                    nc.sync.dma_start(out=u_ch, in_=u_t[:, :, c0 : c0 + ch])
                    t0 = c0 // P
                    v_ch = vpool.tile([P, cht, D], f16, tag="v")
                    nc.scalar.dma_start(out=v_ch, in_=v_p[:, t0 : t0 + cht, :])

                    for st in range(0, cht, G):
                        g = min(G, cht - st)
                        if do_pe:
                            ps = ppool.tile([P, g, D], f32, tag="ps")
                            for j in range(g):
                                sl = slice((st + j) * P, (st + j + 1) * P)
                                nc.tensor.matmul(
                                    ps[:, j, :], u_ch[:, 0, sl], w_sb[:, 0, :],
                                    start=True, stop=False,
                                )
                                nc.tensor.matmul(
                                    ps[:, j, :], u_ch[:, 1, sl], w_sb[:, 1, :],
                                    start=False, stop=True,
                                )
                        elif do_dve:
                            ps = static_ps[gi % 2][:, :g, :]
                        gt = t0 + st
                        if do_dve and SCAN_MODE:
                            cum = spool.tile([P, g, D], f32, tag="cum")
                            nc.vector._custom_dve(
                                scan_op,
                                out=cum.rearrange("p g d -> p (g d)"),
                                in0=ps.rearrange("p g d -> p (g d)"),
                                in1=v_ch[:, st : st + g, :].rearrange(
                                    "p g d -> p (g d)"
                                ),
                            )
                            # boundary column -> ends[:, gi, 1:1+g] (ScalarE)
                            nc.scalar.copy(
                                out=ends[:, gi, 1 : 1 + g],
                                in_=cum[:, :, D - 1 : D],
                            )
                            # per-tile sums = adjacent differences (VectorE)
                            nc.vector.tensor_tensor(
                                out=s_buf[:, gt : gt + g],
                                in0=ends[:, gi, 1 : 1 + g],
                                in1=ends[:, gi, 0:g],
                                op=mybir.AluOpType.subtract,
                            )
                        elif do_dve:
                            scr = spool.tile([P, g, D], f32, tag="scr")
                            for j in range(g):
                                nc.vector.affine_mul_reduce(
                                    out=scr[:, j, :],
                                    accum_out=s_buf[:, gt + j : gt + j + 1],
                                    in0=ps[:, j, :],
                                    in1=v_ch[:, st + j, :],
                                    scale=1.0,
                                    bias=0.0,
                                )
                        gi += 1
                    c0 += ch

                # incremental sigmoid + output drain
                if do_dve:
                    out_pt = out.rearrange("(p t) -> p t", p=P)
                    n_blk = 4
                    bnd = [round(i * n_tiles / n_blk) for i in range(n_blk + 1)]
                    for b0, b1 in zip(bnd[:-1], bnd[1:]):
                        if b1 > b0:
                            nc.scalar.activation(
                                out=sig_buf[:, b0:b1],
                                in_=s_buf[:, b0:b1],
                                func=mybir.ActivationFunctionType.Sigmoid,
                            )
                            nc.sync.dma_start(
                                out=out_pt[:, b0:b1], in_=sig_buf[:, b0:b1]
                            )

    nc.compile()
    return nc


def _get_program(n_pad: int):
    if n_pad not in _PROGRAM_CACHE:
        _PROGRAM_CACHE[n_pad] = _build_program(n_pad)
    return _PROGRAM_CACHE[n_pad]


def _prep(u, v, weights, type_idx):
    """Group rows by type, pad, cast fp16, build per-core input maps."""
    u = np.ascontiguousarray(np.asarray(u, dtype=np.float32))
    v = np.ascontiguousarray(np.asarray(v, dtype=np.float32))
    weights = np.ascontiguousarray(np.asarray(weights, dtype=np.float32))
    ti = np.asarray(type_idx).astype(np.int64).ravel()

    n, d = u.shape
    k = weights.shape[0]
    assert d == D and k == N_CORES

    order = np.argsort(ti, kind="stable")
    counts = np.bincount(ti, minlength=k)
    offsets = np.concatenate(([0], np.cumsum(counts)))
    n_pad = max(P, int(math.ceil(counts.max() / P)) * P)
    n_tiles = n_pad // P

    u16 = u.astype(np.float16)
    v16 = v.astype(np.float16)

    in_maps = []
    core_rows = []
    for c in range(N_CORES):
        rows = order[offsets[c] : offsets[c + 1]]
        core_rows.append(rows)
        cnt = len(rows)
        # u_t[p, h, n] = u[n, h*128+p]
        u_t = np.zeros((P, 2, n_pad), dtype=np.float16)
        ut = u16[rows].T.reshape(2, P, cnt)  # [h, p, n]
        u_t[:, :, :cnt] = ut.transpose(1, 0, 2)
        # v_p[p, t, e] = v[t*128+p, e]
        v_pad = np.zeros((n_pad, D), dtype=np.float16)
        v_pad[:cnt] = v16[rows]
        v_pc = v_pad.reshape(n_tiles, P, D).transpose(1, 0, 2)
        # w_p[p, h, e] = W[h*128+p, e]
        w16 = weights[c].astype(np.float16)
        w_pc = w16.reshape(2, P, D).transpose(1, 0, 2)
        in_maps.append(
            {
                "u_t": u_t,
                "v_p": np.ascontiguousarray(v_pc),
                "w_p": np.ascontiguousarray(w_pc),
            }
        )
    return in_maps, core_rows, n_pad


def _run(u, v, weights, type_idx, trace=False):
    from concourse import bass_utils
    from concourse.bass_interp import get_hw_module

    n = np.asarray(u).shape[0]
    in_maps, core_rows, n_pad = _prep(u, v, weights, type_idx)
    n_tiles = n_pad // P

    nc = _get_program(n_pad)
    old_m = nc.m
    nc.m = get_hw_module(nc.m)
    try:
        res = bass_utils.run_bass_kernel_spmd(
            nc, in_maps, core_ids=list(range(N_CORES)), trace=trace
        )
    finally:
        nc.m = old_m

    final = np.empty((n,), dtype=np.float32)
    for c in range(N_CORES):
        arr = np.asarray(res.results[c]["out"]).reshape(P, n_tiles)
        per_row = arr.T.reshape(-1)[: len(core_rows[c])]
        final[core_rows[c]] = per_row
    return final, res


def kernel(**inputs) -> np.ndarray:
    out, _ = _run(
        inputs["u_hidden"],
        inputs["v_hidden"],
        inputs["weights"],
        inputs["type_idx"],
        trace=False,
    )
    return out
